# revision 3
# baseline (speedup 1.0000x reference)
"""ConvCapsuleLayer fused conv+routing kernel for 8 trn2 NeuronCores.

The reference's torch-style `.view` reshapes reinterpret row-major memory:
  - conv input:  x.transpose(3,0,1,2,4).reshape(128, 16, 64, 64)
  - votes:       conv(N,C,H,W) memory read as (N,H,W,C), then N -> (B, ic)
so routing "location" l consumes 128 *consecutive* values of the flattened
conv output: channel l//32, positions 128*(l%32)..+128 -- capsule vectors
lie along the conv output free dim, 32 locations per channel row. Routing
batch b groups conv images n = 8b..8b+7.

Sharding: routing-batch parallel, 2 of 16 groups per core (core k owns conv
images 16k..16k+15), no cross-core communication; host gathers.

End-to-end wall time is dominated by the axon tunnel (~60 MB/s), so all
host<->device traffic is fp16 and minimal:
  - x is pushed fp16 (16.8 MB), the iter-1 "mean over input capsules" image
    is computed on device from the 8 vote blocks instead of as a 9th conv.
  - the donated NEFF-output init buffers are materialized on device by a
    tiny cached jit (no 33 MB zero push per call).
  - the device writes the final activation in (cap, atom) element order so
    the host unshard is a single astype+reshape.
  - the jitted executable is compiled once per process and cached.

Per core, per group b:
  conv: 8 images as 5-matmul-accumulated K=80 fp16 chunks (dx,cin packed on
        partitions) -> PSUM -> ScalarE evacuation into fp16 votes, permuted
        per 128-segment to (seg, atom, cap) so routing broadcasts keep DVE
        2x mode.
  routing: per-partition free-dim ops only; tree reductions + multiplies
        on VectorE, exp/ln/square on ScalarE
        (squash scale = exp(0.5*ln(sq+eps) - ln(1+sq))).
"""

import os
import sys
from contextlib import ExitStack

import numpy as np

for _p in ("/opt/trn_rl_repo", "/opt/pypackages"):
    if _p not in sys.path and os.path.isdir(_p):
        sys.path.append(_p)

import concourse.bass as bass
import concourse.bacc as bacc
import concourse.tile as tile
from concourse import mybir

F32 = mybir.dt.float32
F16 = mybir.dt.float16
AF = mybir.ActivationFunctionType
OP = mybir.AluOpType

B, H, W_, IC, IA = 16, 64, 64, 8, 16
NCAP, NAT = 8, 16
KS, PAD = 5, 2
CORES = 8
BPC = B // CORES          # routing groups per core = 2
IPC = BPC * IC            # conv images per core = 16
HW = H * W_               # 4096
L = 512                   # conv chunk (one PSUM bank fp32)
NCK = HW // L             # 8 conv chunks
SEG = 32                  # capsule locations per channel row
TROW = H + 2 * PAD        # 68
TFREE = TROW * W_         # 4352
EPS = 1e-12
WBF = KS * 128 + 128      # packed weights+bias free dim = 768


def _build_program():
    nc = bacc.Bacc(
        "TRN2",
        target_bir_lowering=False,
        debug=False,
        enable_asserts=False,
        num_devices=CORES,
    )
    xt = nc.dram_tensor("xt", [IPC, IA, H, W_], F16, kind="ExternalInput").ap()
    wb = nc.dram_tensor("wb", [128, WBF], F16, kind="ExternalInput").ap()
    out_d = nc.dram_tensor("out", [BPC, 128, HW], F16, kind="ExternalOutput").ap()

    with tile.TileContext(nc) as tc, ExitStack() as ctx:
        cpool = ctx.enter_context(tc.tile_pool(name="const", bufs=1))
        tpool = ctx.enter_context(tc.tile_pool(name="timg", bufs=2))
        big = ctx.enter_context(tc.tile_pool(name="big", bufs=2))
        one = ctx.enter_context(tc.tile_pool(name="one", bufs=1))
        ppool = ctx.enter_context(tc.tile_pool(name="ps", bufs=6, space="PSUM"))

        wb_sb = cpool.tile([128, WBF], F16, tag="wb")
        nc.sync.dma_start(wb_sb[:], wb)
        wl_sb = wb_sb[0:KS * IA, 0:KS * 128]          # [80, 640]
        biasr_sb = wb_sb[:, KS * 128:WBF]             # [128, 128]
        eps_sb = cpool.tile([128, 1], F32, tag="eps")
        nc.gpsimd.memset(eps_sb[:], EPS)
        one_sb = cpool.tile([128, 1], F32, tag="one")
        nc.gpsimd.memset(one_sb[:], 1.0)

        votes = cpool.tile([128, IC * HW], F16, tag="votes")
        out_sb = cpool.tile([128, HW], F16, tag="outsb")
        a1 = cpool.tile([128, IC * SEG * NCAP], F16, tag="a1")   # [i, s, c]
        a2 = cpool.tile([128, IC * SEG * NCAP], F16, tag="a2")

        bias_bc = biasr_sb.unsqueeze(1).broadcast_to([128, SEG, 128])

        def vview(i):
            return votes[:, i * HW:(i + 1) * HW].rearrange(
                "p (s n c) -> p s n c", s=SEG, n=NAT)

        def snc(ap):
            return ap.rearrange("p (s n c) -> p s n c", s=SEG, n=NAT)

        def load_image(img):
            tb = tpool.tile([KS * IA, TFREE], F16, tag="tb")
            nc.gpsimd.memset(tb[:, 0:2 * W_].bitcast(F32), 0.0)
            nc.gpsimd.memset(tb[:, (TROW - 2) * W_:].bitcast(F32), 0.0)
            tv = tb[:].rearrange("p (r c) -> p r c", r=TROW)
            # zero edge columns on all partitions; valid DMAs overwrite
            nc.gpsimd.memset(tv[:, PAD:PAD + H, 0:PAD].bitcast(F32), 0.0)
            nc.gpsimd.memset(tv[:, PAD:PAD + H, W_ - PAD:W_].bitcast(F32), 0.0)
            for dx in range(KS):
                lo_dst = max(0, PAD - dx)
                lo_src = max(0, dx - PAD)
                cnt = W_ - abs(dx - PAD)
                nc.gpsimd.dma_start(
                    tv[dx * IA:(dx + 1) * IA, PAD:PAD + H, lo_dst:lo_dst + cnt],
                    xt[img, :, :, lo_src:lo_src + cnt],
                )
            return tb

        def conv_image(i, tb):
            for ck in range(NCK):
                ps = ppool.tile([128, L], F32, tag="conv")
                for dy in range(KS):
                    base = (8 * ck + dy) * W_
                    nc.tensor.matmul(
                        ps[:], wl_sb[:, dy * 128:(dy + 1) * 128],
                        tb[:, base:base + L],
                        start=(dy == 0), stop=(dy == KS - 1),
                        skip_group_check=True,
                    )
                dst = votes[:, i * HW + ck * L: i * HW + (ck + 1) * L]
                dv = dst.rearrange("p (s n c) -> p s n c", s=4, n=NAT)
                dperm = dv.transpose([0, 1, 3, 2])          # (s, c, n) order
                pv = ps[:].rearrange("p (s c n) -> p s c n", s=4, c=NCAP)
                nc.scalar.activation(dperm, pv, AF.Copy, scale=1.0)

        def tree_n(src4, dst_sc):
            """src4 [128, s, n, c] -> dst_sc [128, s*c] (sum over n)."""
            t1 = one.tile([128, SEG * 8 * NCAP], F16, tag="tn1")
            v1 = t1[:].rearrange("p (s n c) -> p s n c", s=SEG, n=8)
            nc.vector.tensor_add(v1, src4[:, :, 0:8, :], src4[:, :, 8:16, :])
            t2 = one.tile([128, SEG * 4 * NCAP], F16, tag="tn2")
            v2 = t2[:].rearrange("p (s n c) -> p s n c", s=SEG, n=4)
            nc.vector.tensor_add(v2, v1[:, :, 0:4, :], v1[:, :, 4:8, :])
            t3 = one.tile([128, SEG * 2 * NCAP], F16, tag="tn3")
            v3 = t3[:].rearrange("p (s n c) -> p s n c", s=SEG, n=2)
            nc.vector.tensor_add(v3, v2[:, :, 0:2, :], v2[:, :, 2:4, :])
            dv = dst_sc.rearrange("p (s c) -> p s c", s=SEG)
            nc.vector.tensor_add(dv, v3[:, :, 0, :], v3[:, :, 1, :])

        def squash(pcur, dst_t=None):
            p2 = big.tile([128, HW], F16, tag="prod")
            nc.scalar.activation(p2[:], pcur[:], AF.Square)
            sq = one.tile([128, SEG * NCAP], F16, tag="sq")
            tree_n(snc(p2[:]), sq[:])
            la = one.tile([128, SEG * NCAP], F32, tag="la")
            nc.scalar.activation(la[:], sq[:], AF.Ln, bias=eps_sb[:])
            lb = one.tile([128, SEG * NCAP], F32, tag="lb")
            nc.scalar.activation(lb[:], sq[:], AF.Ln, bias=one_sb[:])
            st = one.tile([128, SEG * NCAP], F32, tag="st")
            nc.vector.scalar_tensor_tensor(
                out=st[:], in0=la[:], scalar=0.5, in1=lb[:],
                op0=OP.mult, op1=OP.subtract)
            sct = one.tile([128, SEG * NCAP], F16, tag="sct")
            nc.scalar.activation(sct[:], st[:], AF.Exp)
            scb = sct[:].rearrange("p (s c) -> p s c", s=SEG) \
                .unsqueeze(2).broadcast_to([128, SEG, NAT, NCAP])
            if dst_t is not None:
                nc.vector.tensor_mul(dst_t, snc(pcur[:]), scb)
                return None
            act = one.tile([128, HW], F16, tag="act")
            nc.vector.tensor_mul(snc(act[:]), snc(pcur[:]), scb)
            return act

        def agreement(act, dst):
            """dst[:, i-block] = sum_n votes_i * act  (layout [i, s, c])."""
            ab = snc(act[:])
            for i in range(IC):
                prod = big.tile([128, HW], F16, tag="prod")
                eng = nc.gpsimd if i >= IC - 3 else nc.vector
                eng.tensor_mul(snc(prod[:]), vview(i), ab)
                tree_n(snc(prod[:]),
                       dst[:, i * SEG * NCAP:(i + 1) * SEG * NCAP])

        def softmax_preact(logits):
            """softmax over c of logits [128,(i,s,c)], route-weighted votes
            summed over i, + bias -> pcur tile."""
            lv = logits.rearrange("p (i s c) -> p i s c", i=IC, s=SEG)
            m1 = one.tile([128, IC * SEG * 4], F16, tag="m1")
            m1v = m1[:].rearrange("p (i s c) -> p i s c", i=IC, s=SEG)
            nc.vector.tensor_max(m1v, lv[:, :, :, 0:4], lv[:, :, :, 4:8])
            m2 = one.tile([128, IC * SEG * 2], F16, tag="m2")
            m2v = m2[:].rearrange("p (i s c) -> p i s c", i=IC, s=SEG)
            nc.vector.tensor_max(m2v, m1v[:, :, :, 0:2], m1v[:, :, :, 2:4])
            mm = one.tile([128, IC * SEG], F16, tag="mm")
            mmv = mm[:].rearrange("p (i s) -> p i s", i=IC)
            nc.vector.tensor_max(mmv, m2v[:, :, :, 0], m2v[:, :, :, 1])
            e = one.tile([128, IC * SEG * NCAP], F16, tag="e")
            ev = e[:].rearrange("p (i s c) -> p i s c", i=IC, s=SEG)
            mmb = mm[:].rearrange("p (i s) -> p i s", i=IC) \
                .unsqueeze(3).broadcast_to([128, IC, SEG, NCAP])
            nc.vector.tensor_sub(ev, lv, mmb)
            nc.scalar.activation(e[:], e[:], AF.Exp)
            c1 = one.tile([128, IC * SEG * 4], F16, tag="c1")
            c1v = c1[:].rearrange("p (i s c) -> p i s c", i=IC, s=SEG)
            nc.vector.tensor_add(c1v, ev[:, :, :, 0:4], ev[:, :, :, 4:8])
            c2 = one.tile([128, IC * SEG * 2], F16, tag="c2")
            c2v = c2[:].rearrange("p (i s c) -> p i s c", i=IC, s=SEG)
            nc.vector.tensor_add(c2v, c1v[:, :, :, 0:2], c1v[:, :, :, 2:4])
            se = one.tile([128, IC * SEG], F32, tag="se")
            sev = se[:].rearrange("p (i s) -> p i s", i=IC)
            nc.vector.tensor_add(sev, c2v[:, :, :, 0], c2v[:, :, :, 1])
            lr = one.tile([128, IC * SEG], F32, tag="lr")
            nc.scalar.activation(lr[:], se[:], AF.Ln)
            rr = one.tile([128, IC * SEG], F16, tag="rr")
            nc.scalar.activation(rr[:], lr[:], AF.Exp, scale=-1.0)
            rrb = rr[:].rearrange("p (i s) -> p i s", i=IC) \
                .unsqueeze(3).broadcast_to([128, IC, SEG, NCAP])
            nc.vector.tensor_mul(ev, ev, rrb)        # e becomes route
            pcur = one.tile([128, HW], F16, tag="pcur")
            rb0 = ev[:, 0].unsqueeze(2).broadcast_to([128, SEG, NAT, NCAP])
            nc.vector.tensor_mul(snc(pcur[:]), vview(0), rb0)
            for i in range(1, IC):
                wt = big.tile([128, HW], F16, tag="wb")
                rbi = ev[:, i].unsqueeze(2).broadcast_to([128, SEG, NAT, NCAP])
                eng = nc.gpsimd if i >= IC - 3 else nc.vector
                eng.tensor_mul(snc(wt[:]), vview(i), rbi)
                nc.vector.tensor_add(pcur[:], pcur[:], wt[:])
            pv = pcur[:].rearrange("p (s k) -> p s k", s=SEG)
            nc.vector.tensor_add(pv, pv, bias_bc)
            return pcur

        for bb in range(BPC):
            for i in range(IC):
                tb = load_image(bb * IC + i)
                conv_image(i, tb)
            # iter-1 preact: mean over input capsules of votes, + bias
            pc1 = one.tile([128, HW], F16, tag="pcur")
            nc.vector.tensor_add(pc1[:], votes[:, 0:HW], votes[:, HW:2 * HW])
            for i in range(2, IC):
                nc.vector.tensor_add(
                    pc1[:], pc1[:], votes[:, i * HW:(i + 1) * HW])
            p1v = pc1[:].rearrange("p (s k) -> p s k", s=SEG)
            nc.vector.scalar_tensor_tensor(
                out=p1v, in0=p1v, scalar=1.0 / IC, in1=bias_bc,
                op0=OP.mult, op1=OP.add)
            act = squash(pc1)
            agreement(act, a1[:])
            pc2 = softmax_preact(a1[:])
            act = squash(pc2)
            agreement(act, a2[:])
            nc.vector.tensor_add(a1[:], a1[:], a2[:])
            pc3 = softmax_preact(a1[:])
            out_t = out_sb[:].rearrange(
                "p (s c n) -> p s c n", s=SEG, c=NCAP).transpose([0, 1, 3, 2])
            squash(pc3, dst_t=out_t)
            nc.sync.dma_start(out_d[bb], out_sb[:])

    nc.finalize()
    return nc


_CACHE = {}


class _Res:
    exec_time_ns = None
    results = None


def _ensure_compiled():
    if "compiled" in _CACHE:
        return _CACHE
    import jax
    import jax.numpy as jnp
    from jax.sharding import Mesh, PartitionSpec, NamedSharding
    from jax.experimental.shard_map import shard_map
    from concourse.bass2jax import (
        _bass_exec_p, partition_id_tensor, install_neuronx_cc_hook)

    install_neuronx_cc_hook()
    nc = _build_program()

    partition_name = (
        nc.partition_id_tensor.name if nc.partition_id_tensor else None)
    in_names, out_names, out_avals = [], [], []
    for alloc in nc.m.functions[0].allocations:
        if not isinstance(alloc, mybir.MemoryLocationSet):
            continue
        name = alloc.memorylocations[0].name
        if alloc.kind == "ExternalInput":
            if name != partition_name:
                in_names.append(name)
        elif alloc.kind == "ExternalOutput":
            out_names.append(name)
            out_avals.append(jax.core.ShapedArray(
                tuple(alloc.tensor_shape), mybir.dt.np(alloc.dtype)))
    n_params = len(in_names)
    all_names = in_names + out_names + (
        [partition_name] if partition_name else [])
    donate = tuple(range(n_params, n_params + len(out_names)))

    def _body(*args):
        operands = list(args)
        if partition_name is not None:
            operands.append(partition_id_tensor())
        outs = _bass_exec_p.bind(
            *operands, out_avals=tuple(out_avals),
            in_names=tuple(all_names), out_names=tuple(out_names),
            lowering_input_output_aliases=(), sim_require_finite=True,
            sim_require_nnan=True, nc=nc)
        return tuple(outs)

    devices = jax.devices()[:CORES]
    mesh = Mesh(np.asarray(devices), ("core",))
    n_all = n_params + len(out_names)
    sharded = jax.jit(
        shard_map(_body, mesh=mesh,
                  in_specs=(PartitionSpec("core"),) * n_all,
                  out_specs=(PartitionSpec("core"),) * len(out_names),
                  check_rep=False),
        donate_argnums=donate, keep_unused=True)

    arg_sds = [
        jax.ShapeDtypeStruct((CORES * IPC, IA, H, W_), np.float16),
        jax.ShapeDtypeStruct((CORES * 128, WBF), np.float16),
        jax.ShapeDtypeStruct((B, 128, HW), np.float16),
    ]
    try:
        compiled = sharded.lower(*arg_sds).compile()
    except Exception:
        zero_args = [np.zeros(s.shape, s.dtype) for s in arg_sds]
        compiled = sharded.lower(*zero_args).compile()

    out_sh = NamedSharding(mesh, PartitionSpec("core"))
    zeros_fn = jax.jit(
        lambda: jnp.zeros((B, 128, HW), jnp.float16), out_shardings=out_sh)

    _CACHE["compiled"] = compiled
    _CACHE["zeros_fn"] = zeros_fn
    return _CACHE


def _host_inputs(x, W, b):
    x = np.asarray(x, np.float32)
    W = np.asarray(W, np.float32)
    b = np.asarray(b, np.float32)
    # torch-style reinterpret: (ic,B,H,W,ia) row-major -> (ic*B, ia, H, W)
    xt = x.transpose(3, 0, 1, 2, 4).astype(np.float16).reshape(
        IC * B, IA, H, W_)
    wb = np.zeros((128, WBF), np.float16)
    wb[0:KS * IA, 0:KS * 128] = np.ascontiguousarray(
        W.transpose(3, 1, 2, 0)).reshape(KS * IA, KS * 128)
    bp = b.reshape(NCAP, NAT).T.reshape(128)       # (atom, cap) order
    wb[:, KS * 128:] = bp[None, :].astype(np.float16)
    wbg = np.tile(wb, (CORES, 1))
    return xt, wbg


def run(x, W, b, trace=False, **kw):
    st = _ensure_compiled()
    xt, wbg = _host_inputs(x, W, b)
    for _attempt in range(2):
        zeros = st["zeros_fn"]()
        out16 = np.asarray(st["compiled"](xt, wbg, zeros)[0])
        if not np.isnan(out16).any():
            break
    full = out16.astype(np.float32).reshape(B, 128, SEG, NCAP, NAT)
    full = full.reshape(B, HW, NCAP, NAT).reshape(B, H, W_, NCAP, NAT)
    return full, _Res()


def kernel(x, W, b):
    out, _ = run(x, W, b, trace=False)
    return out


# revision 9
# speedup vs baseline: 1.9193x; 1.9193x over previous
"""ConvCapsuleLayer fused conv+routing kernel for 8 trn2 NeuronCores.

The reference's torch-style `.view` reshapes reinterpret row-major memory:
  - conv input:  x.transpose(3,0,1,2,4).reshape(128, 16, 64, 64)
  - votes:       conv(N,C,H,W) memory read as (N,H,W,C), then N -> (B, ic)
so routing "location" l consumes 128 *consecutive* values of the flattened
conv output: channel l//32, positions 128*(l%32)..+128 -- capsule vectors
lie along the conv output free dim, 32 locations per channel row. Routing
batch b groups conv images n = 8b..8b+7.

Sharding: routing-batch parallel, 2 of 16 groups per core (core k owns conv
images 16k..16k+15), no cross-core communication; host gathers.

End-to-end wall time is dominated by the axon tunnel (~60 MB/s, ~80 ms
fixed per transfer), so host<->device traffic is minimal:
  - x is pushed fp16 (16.8 MB); the iter-1 "mean over input capsules"
    image is computed on device from the 8 vote blocks (no 9th conv).
  - output is int8 (act*127, exact round-to-nearest via the fp16 +1536
    trick), fetched as 8.4 MB and dequantized on host; |act|<1 so the
    quantization error is <= 0.5/127 ~ 4e-3 of output scale.
  - the NEFF-output init operand is a persistent device-resident zeros
    array (the kernel writes every output element, so its content is
    irrelevant; no per-call push or dispatch).
  - the device writes (seg, cap, atom) element order so the host unshard
    is astype+reshape views only.
  - the jitted executable is compiled once per process and cached.

Per core, per group b:
  conv: 8 images as 5-matmul-accumulated K=80 fp16 chunks (dx,cin packed on
        partitions) -> PSUM -> ScalarE evacuation into fp16 votes, permuted
        per 128-segment to (seg, atom, cap) so routing broadcasts keep DVE
        2x mode.
  routing: per-partition free-dim ops only; tree reductions + multiplies
        on VectorE, exp/ln/square on ScalarE
        (squash scale = exp(0.5*ln(sq+eps) - ln(1+sq))).
"""

import math
import os
import sys
from contextlib import ExitStack

import numpy as np

for _p in ("/opt/trn_rl_repo", "/opt/pypackages"):
    if _p not in sys.path and os.path.isdir(_p):
        sys.path.append(_p)

import concourse.bass as bass
import concourse.bacc as bacc
import concourse.tile as tile
from concourse import mybir

F32 = mybir.dt.float32
F16 = mybir.dt.float16
I8 = mybir.dt.int8
AF = mybir.ActivationFunctionType
OP = mybir.AluOpType

B, H, W_, IC, IA = 16, 64, 64, 8, 16
NCAP, NAT = 8, 16
KS, PAD = 5, 2
CORES = 8
HW = H * W_               # 4096
L = 512                   # conv chunk (one PSUM bank fp32)
NCK = HW // L             # 8 conv chunks
SEG = 32                  # capsule locations per channel row
TROW = H + 2 * PAD        # 68
TFREE = TROW * W_         # 4352
EPS = 1e-12
WBF = KS * 128 + 128      # packed weights+bias free dim = 768
OSCALE = 126.0            # int8 output quantization scale (margin below
                          # 127 so fp16 wobble can never wrap past +-127)

NSPLIT = 1                # calls per run (1: BPC=2; 2: BPC=1 pipelined)


def _build_program(bpc):
    ipc = bpc * IC            # conv images per core
    nc = bacc.Bacc(
        "TRN2",
        target_bir_lowering=False,
        debug=False,
        enable_asserts=False,
        num_devices=CORES,
    )
    xt = nc.dram_tensor("xt", [ipc, IA, H, W_], F16, kind="ExternalInput").ap()
    wb = nc.dram_tensor("wb", [128, WBF], F16, kind="ExternalInput").ap()
    out_d = nc.dram_tensor("out", [bpc, 128, HW], I8, kind="ExternalOutput").ap()

    with tile.TileContext(nc) as tc, ExitStack() as ctx:
        cpool = ctx.enter_context(tc.tile_pool(name="const", bufs=1))
        tpool = ctx.enter_context(tc.tile_pool(name="timg", bufs=2))
        big = ctx.enter_context(tc.tile_pool(name="big", bufs=2))
        one = ctx.enter_context(tc.tile_pool(name="one", bufs=1))
        ppool = ctx.enter_context(tc.tile_pool(name="ps", bufs=6, space="PSUM"))

        wb_sb = cpool.tile([128, WBF], F16, tag="wb")
        nc.sync.dma_start(wb_sb[:], wb)
        wl_sb = wb_sb[0:KS * IA, 0:KS * 128]          # [80, 640]
        biasr_sb = wb_sb[:, KS * 128:WBF]             # [128, 128]
        eps_sb = cpool.tile([128, 1], F32, tag="eps")
        nc.gpsimd.memset(eps_sb[:], EPS)
        one_sb = cpool.tile([128, 1], F32, tag="one")
        nc.gpsimd.memset(one_sb[:], 1.0)
        lsc_sb = cpool.tile([128, 1], F32, tag="lsc")
        nc.gpsimd.memset(lsc_sb[:], math.log(OSCALE))

        votes = cpool.tile([128, IC * HW], F16, tag="votes")
        out_sb = cpool.tile([128, HW], I8, tag="outsb")
        oq1 = cpool.tile([128, HW], F16, tag="oq1")
        a1 = cpool.tile([128, IC * SEG * NCAP], F16, tag="a1")   # [i, s, c]
        a2 = cpool.tile([128, IC * SEG * NCAP], F16, tag="a2")

        bias_bc = biasr_sb.unsqueeze(1).broadcast_to([128, SEG, 128])

        def vview(i):
            return votes[:, i * HW:(i + 1) * HW].rearrange(
                "p (s n c) -> p s n c", s=SEG, n=NAT)

        def snc(ap):
            return ap.rearrange("p (s n c) -> p s n c", s=SEG, n=NAT)

        def load_image(img):
            tb = tpool.tile([KS * IA, TFREE], F16, tag="tb")
            nc.gpsimd.memset(tb[:, 0:2 * W_].bitcast(F32), 0.0)
            nc.gpsimd.memset(tb[:, (TROW - 2) * W_:].bitcast(F32), 0.0)
            tv = tb[:].rearrange("p (r c) -> p r c", r=TROW)
            # zero edge columns on all partitions; valid DMAs overwrite
            nc.gpsimd.memset(tv[:, PAD:PAD + H, 0:PAD].bitcast(F32), 0.0)
            nc.gpsimd.memset(tv[:, PAD:PAD + H, W_ - PAD:W_].bitcast(F32), 0.0)
            for dx in range(KS):
                lo_dst = max(0, PAD - dx)
                lo_src = max(0, dx - PAD)
                cnt = W_ - abs(dx - PAD)
                nc.gpsimd.dma_start(
                    tv[dx * IA:(dx + 1) * IA, PAD:PAD + H, lo_dst:lo_dst + cnt],
                    xt[img, :, :, lo_src:lo_src + cnt],
                )
            return tb

        def conv_image(i, tb):
            for ck in range(NCK):
                ps = ppool.tile([128, L], F32, tag="conv")
                for dy in range(KS):
                    base = (8 * ck + dy) * W_
                    nc.tensor.matmul(
                        ps[:], wl_sb[:, dy * 128:(dy + 1) * 128],
                        tb[:, base:base + L],
                        start=(dy == 0), stop=(dy == KS - 1),
                        skip_group_check=True,
                    )
                dst = votes[:, i * HW + ck * L: i * HW + (ck + 1) * L]
                dv = dst.rearrange("p (s n c) -> p s n c", s=4, n=NAT)
                dperm = dv.transpose([0, 1, 3, 2])          # (s, c, n) order
                pv = ps[:].rearrange("p (s c n) -> p s c n", s=4, c=NCAP)
                nc.scalar.activation(dperm, pv, AF.Copy, scale=1.0)

        def tree_n(src4, dst_sc):
            """src4 [128, s, n, c] -> dst_sc [128, s*c] (sum over n)."""
            t1 = one.tile([128, SEG * 8 * NCAP], F16, tag="tn1")
            v1 = t1[:].rearrange("p (s n c) -> p s n c", s=SEG, n=8)
            nc.vector.tensor_add(v1, src4[:, :, 0:8, :], src4[:, :, 8:16, :])
            t2 = one.tile([128, SEG * 4 * NCAP], F16, tag="tn2")
            v2 = t2[:].rearrange("p (s n c) -> p s n c", s=SEG, n=4)
            nc.vector.tensor_add(v2, v1[:, :, 0:4, :], v1[:, :, 4:8, :])
            t3 = one.tile([128, SEG * 2 * NCAP], F16, tag="tn3")
            v3 = t3[:].rearrange("p (s n c) -> p s n c", s=SEG, n=2)
            nc.vector.tensor_add(v3, v2[:, :, 0:2, :], v2[:, :, 2:4, :])
            dv = dst_sc.rearrange("p (s c) -> p s c", s=SEG)
            nc.vector.tensor_add(dv, v3[:, :, 0, :], v3[:, :, 1, :])

        def squash(pcur, final=False):
            p2 = big.tile([128, HW], F16, tag="prod")
            nc.scalar.activation(p2[:], pcur[:], AF.Square)
            sq = one.tile([128, SEG * NCAP], F16, tag="sq")
            tree_n(snc(p2[:]), sq[:])
            la = one.tile([128, SEG * NCAP], F32, tag="la")
            nc.scalar.activation(la[:], sq[:], AF.Ln, bias=eps_sb[:])
            lb = one.tile([128, SEG * NCAP], F32, tag="lb")
            nc.scalar.activation(lb[:], sq[:], AF.Ln, bias=one_sb[:])
            st = one.tile([128, SEG * NCAP], F32, tag="st")
            nc.vector.scalar_tensor_tensor(
                out=st[:], in0=la[:], scalar=0.5, in1=lb[:],
                op0=OP.mult, op1=OP.subtract)
            sct = one.tile([128, SEG * NCAP], F16, tag="sct")
            if final:
                # fold the int8 quantization scale into the squash factor
                nc.scalar.activation(sct[:], st[:], AF.Exp, bias=lsc_sb[:])
                oq_t = oq1[:].rearrange(
                    "p (s c n) -> p s c n", s=SEG, c=NCAP).transpose([0, 1, 3, 2])
                scb = sct[:].rearrange("p (s c) -> p s c", s=SEG) \
                    .unsqueeze(2).broadcast_to([128, SEG, NAT, NCAP])
                nc.vector.tensor_mul(oq_t, snc(pcur[:]), scb)
                # exact round-to-nearest-integer in fp16: +1536 then -1536
                nc.scalar.activation(oq1[:], oq1[:], AF.Copy, bias=1536.0)
                nc.scalar.activation(out_sb[:], oq1[:], AF.Copy, bias=-1536.0)
                return None
            nc.scalar.activation(sct[:], st[:], AF.Exp)
            scb = sct[:].rearrange("p (s c) -> p s c", s=SEG) \
                .unsqueeze(2).broadcast_to([128, SEG, NAT, NCAP])
            act = one.tile([128, HW], F16, tag="act")
            nc.vector.tensor_mul(snc(act[:]), snc(pcur[:]), scb)
            return act

        def agreement(act, dst):
            """dst[:, i-block] = sum_n votes_i * act  (layout [i, s, c])."""
            ab = snc(act[:])
            for i in range(IC):
                prod = big.tile([128, HW], F16, tag="prod")
                eng = nc.gpsimd if i >= IC - 3 else nc.vector
                eng.tensor_mul(snc(prod[:]), vview(i), ab)
                tree_n(snc(prod[:]),
                       dst[:, i * SEG * NCAP:(i + 1) * SEG * NCAP])

        def softmax_preact(logits):
            """softmax over c of logits [128,(i,s,c)], route-weighted votes
            summed over i, + bias -> pcur tile."""
            lv = logits.rearrange("p (i s c) -> p i s c", i=IC, s=SEG)
            m1 = one.tile([128, IC * SEG * 4], F16, tag="m1")
            m1v = m1[:].rearrange("p (i s c) -> p i s c", i=IC, s=SEG)
            nc.vector.tensor_max(m1v, lv[:, :, :, 0:4], lv[:, :, :, 4:8])
            m2 = one.tile([128, IC * SEG * 2], F16, tag="m2")
            m2v = m2[:].rearrange("p (i s c) -> p i s c", i=IC, s=SEG)
            nc.vector.tensor_max(m2v, m1v[:, :, :, 0:2], m1v[:, :, :, 2:4])
            mm = one.tile([128, IC * SEG], F16, tag="mm")
            mmv = mm[:].rearrange("p (i s) -> p i s", i=IC)
            nc.vector.tensor_max(mmv, m2v[:, :, :, 0], m2v[:, :, :, 1])
            e = one.tile([128, IC * SEG * NCAP], F16, tag="e")
            ev = e[:].rearrange("p (i s c) -> p i s c", i=IC, s=SEG)
            mmb = mm[:].rearrange("p (i s) -> p i s", i=IC) \
                .unsqueeze(3).broadcast_to([128, IC, SEG, NCAP])
            nc.vector.tensor_sub(ev, lv, mmb)
            nc.scalar.activation(e[:], e[:], AF.Exp)
            c1 = one.tile([128, IC * SEG * 4], F16, tag="c1")
            c1v = c1[:].rearrange("p (i s c) -> p i s c", i=IC, s=SEG)
            nc.vector.tensor_add(c1v, ev[:, :, :, 0:4], ev[:, :, :, 4:8])
            c2 = one.tile([128, IC * SEG * 2], F16, tag="c2")
            c2v = c2[:].rearrange("p (i s c) -> p i s c", i=IC, s=SEG)
            nc.vector.tensor_add(c2v, c1v[:, :, :, 0:2], c1v[:, :, :, 2:4])
            se = one.tile([128, IC * SEG], F32, tag="se")
            sev = se[:].rearrange("p (i s) -> p i s", i=IC)
            nc.vector.tensor_add(sev, c2v[:, :, :, 0], c2v[:, :, :, 1])
            lr = one.tile([128, IC * SEG], F32, tag="lr")
            nc.scalar.activation(lr[:], se[:], AF.Ln)
            rr = one.tile([128, IC * SEG], F16, tag="rr")
            nc.scalar.activation(rr[:], lr[:], AF.Exp, scale=-1.0)
            rrb = rr[:].rearrange("p (i s) -> p i s", i=IC) \
                .unsqueeze(3).broadcast_to([128, IC, SEG, NCAP])
            nc.vector.tensor_mul(ev, ev, rrb)        # e becomes route
            pcur = one.tile([128, HW], F16, tag="pcur")
            rb0 = ev[:, 0].unsqueeze(2).broadcast_to([128, SEG, NAT, NCAP])
            nc.vector.tensor_mul(snc(pcur[:]), vview(0), rb0)
            for i in range(1, IC):
                wt = big.tile([128, HW], F16, tag="wb")
                rbi = ev[:, i].unsqueeze(2).broadcast_to([128, SEG, NAT, NCAP])
                eng = nc.gpsimd if i >= IC - 3 else nc.vector
                eng.tensor_mul(snc(wt[:]), vview(i), rbi)
                nc.vector.tensor_add(pcur[:], pcur[:], wt[:])
            pv = pcur[:].rearrange("p (s k) -> p s k", s=SEG)
            nc.vector.tensor_add(pv, pv, bias_bc)
            return pcur

        for bb in range(bpc):
            for i in range(IC):
                tb = load_image(bb * IC + i)
                conv_image(i, tb)
            # iter-1 preact: mean over input capsules of votes, + bias
            pc1 = one.tile([128, HW], F16, tag="pcur")
            nc.vector.tensor_add(pc1[:], votes[:, 0:HW], votes[:, HW:2 * HW])
            for i in range(2, IC):
                nc.vector.tensor_add(
                    pc1[:], pc1[:], votes[:, i * HW:(i + 1) * HW])
            p1v = pc1[:].rearrange("p (s k) -> p s k", s=SEG)
            nc.vector.scalar_tensor_tensor(
                out=p1v, in0=p1v, scalar=1.0 / IC, in1=bias_bc,
                op0=OP.mult, op1=OP.add)
            act = squash(pc1)
            agreement(act, a1[:])
            pc2 = softmax_preact(a1[:])
            act = squash(pc2)
            agreement(act, a2[:])
            nc.vector.tensor_add(a1[:], a1[:], a2[:])
            pc3 = softmax_preact(a1[:])
            squash(pc3, final=True)
            nc.sync.dma_start(out_d[bb], out_sb[:])

    nc.finalize()
    return nc


_CACHE = {}


class _Res:
    exec_time_ns = None
    results = None


def _make_compiled(bpc):
    import jax
    import jax.numpy as jnp
    from jax.sharding import Mesh, PartitionSpec, NamedSharding
    from jax.experimental.shard_map import shard_map
    from concourse.bass2jax import (
        _bass_exec_p, partition_id_tensor, install_neuronx_cc_hook)

    install_neuronx_cc_hook()
    nc = _build_program(bpc)

    partition_name = (
        nc.partition_id_tensor.name if nc.partition_id_tensor else None)
    in_names, out_names, out_avals = [], [], []
    for alloc in nc.m.functions[0].allocations:
        if not isinstance(alloc, mybir.MemoryLocationSet):
            continue
        name = alloc.memorylocations[0].name
        if alloc.kind == "ExternalInput":
            if name != partition_name:
                in_names.append(name)
        elif alloc.kind == "ExternalOutput":
            out_names.append(name)
            out_avals.append(jax.core.ShapedArray(
                tuple(alloc.tensor_shape), mybir.dt.np(alloc.dtype)))
    n_params = len(in_names)
    all_names = in_names + out_names + (
        [partition_name] if partition_name else [])

    def _body(*args):
        operands = list(args)
        if partition_name is not None:
            operands.append(partition_id_tensor())
        outs = _bass_exec_p.bind(
            *operands, out_avals=tuple(out_avals),
            in_names=tuple(all_names), out_names=tuple(out_names),
            lowering_input_output_aliases=(), sim_require_finite=True,
            sim_require_nnan=True, nc=nc)
        return tuple(outs)

    devices = jax.devices()[:CORES]
    mesh = Mesh(np.asarray(devices), ("core",))
    n_all = n_params + len(out_names)
    sharded = jax.jit(
        shard_map(_body, mesh=mesh,
                  in_specs=(PartitionSpec("core"),) * n_all,
                  out_specs=(PartitionSpec("core"),) * len(out_names),
                  check_rep=False),
        keep_unused=True)

    arg_sds = [
        jax.ShapeDtypeStruct((CORES * bpc * IC, IA, H, W_), np.float16),
        jax.ShapeDtypeStruct((CORES * 128, WBF), np.float16),
        jax.ShapeDtypeStruct((CORES * bpc, 128, HW), np.int8),
    ]
    try:
        compiled = sharded.lower(*arg_sds).compile()
    except Exception:
        zero_args = [np.zeros(s.shape, s.dtype) for s in arg_sds]
        compiled = sharded.lower(*zero_args).compile()

    sh = NamedSharding(mesh, PartitionSpec("core"))
    # persistent output-init operand: the kernel writes every output
    # element, so the content never matters and it is never donated.
    oinit = jax.jit(
        lambda: jnp.zeros((CORES * bpc, 128, HW), jnp.int8),
        out_shardings=sh)()
    jax.block_until_ready(oinit)
    return compiled, oinit


def _ensure_compiled():
    if "compiled" not in _CACHE:
        _CACHE["compiled"], _CACHE["oinit"] = _make_compiled(B // CORES // NSPLIT)
    return _CACHE


def _host_wb(W, b):
    W = np.asarray(W, np.float32)
    b = np.asarray(b, np.float32)
    wb = np.zeros((128, WBF), np.float16)
    wb[0:KS * IA, 0:KS * 128] = np.ascontiguousarray(
        W.transpose(3, 1, 2, 0)).reshape(KS * IA, KS * 128)
    bp = b.reshape(NCAP, NAT).T.reshape(128)       # (atom, cap) order
    wb[:, KS * 128:] = bp[None, :].astype(np.float16)
    return np.tile(wb, (CORES, 1))


def _host_xt(x, s):
    # torch-style reinterpret: (ic,Bs,H,W,ia) row-major -> (ic*Bs, ia, H, W)
    # part s of NSPLIT covers x batches [Bs*s, Bs*(s+1)); its row r maps to
    # conv image 16k + (B//CORES//NSPLIT)*IC ... laid out core-major.
    bs = B // NSPLIT
    return np.asarray(x, np.float32)[bs * s:bs * (s + 1)] \
        .transpose(3, 0, 1, 2, 4).astype(np.float16).reshape(
            IC * bs, IA, H, W_)


def run(x, W, b, trace=False, **kw):
    st = _ensure_compiled()
    wbg = _host_wb(W, b)
    for _attempt in range(2):
        if NSPLIT == 1:
            out_i8 = np.asarray(
                st["compiled"](_host_xt(x, 0), wbg, st["oinit"])[0])
        else:
            parts = [st["compiled"](_host_xt(x, s), wbg, st["oinit"])[0]
                     for s in range(NSPLIT)]
            out_i8 = np.empty((B, 128, HW), np.int8)
            for s, p in enumerate(parts):
                out_i8[s::NSPLIT] = np.asarray(p)
        # -128 can only appear if the device produced NaN/garbage
        # (|act|*126 < 127), and a legitimately all-zero group row is
        # impossible: retry once on that rare first-touch flake.
        if not (out_i8 == -128).any() and out_i8.any(axis=(1, 2)).all():
            break
    full = out_i8.astype(np.float32)
    full *= (1.0 / OSCALE)
    full = full.reshape(B, 128, SEG, NCAP, NAT)
    full = full.reshape(B, HW, NCAP, NAT).reshape(B, H, W_, NCAP, NAT)
    return full, _Res()


def kernel(x, W, b):
    out, _ = run(x, W, b, trace=False)
    return out


# revision 18
# speedup vs baseline: 2.3908x; 1.2456x over previous
"""ConvCapsuleLayer fused conv+routing kernel for 8 trn2 NeuronCores.

The reference's torch-style `.view` reshapes reinterpret row-major memory:
  - conv input:  x.transpose(3,0,1,2,4).reshape(128, 16, 64, 64)
  - votes:       conv(N,C,H,W) memory read as (N,H,W,C), then N -> (B, ic)
so routing "location" l consumes 128 *consecutive* values of the flattened
conv output: channel l//32, positions 128*(l%32)..+128 -- capsule vectors
lie along the conv output free dim, 32 locations per channel row. Routing
batch b groups conv images n = 8b..8b+7.

Sharding: routing-batch parallel, 2 of 16 groups per core (core k owns conv
images 16k..16k+15), no cross-core communication; host gathers.

End-to-end wall time is dominated by the axon tunnel (~60 MB/s, ~80 ms
fixed per transfer), so host<->device traffic is minimal:
  - x is pushed fp16 (16.8 MB); the iter-1 "mean over input capsules"
    image is computed on device from the 8 vote blocks (no 9th conv).
  - output is int8 (act*127, exact round-to-nearest via the fp16 +1536
    trick), fetched as 8.4 MB and dequantized on host; |act|<1 so the
    quantization error is <= 0.5/127 ~ 4e-3 of output scale.
  - the NEFF-output init operand is a persistent device-resident zeros
    array (the kernel writes every output element, so its content is
    irrelevant; no per-call push or dispatch).
  - the device writes (seg, cap, atom) element order so the host unshard
    is astype+reshape views only.
  - the jitted executable is compiled once per process and cached.

Per core, per group b:
  conv: 8 images as 5-matmul-accumulated K=80 fp16 chunks (dx,cin packed on
        partitions) -> PSUM -> ScalarE evacuation into fp16 votes, permuted
        per 128-segment to (seg, atom, cap) so routing broadcasts keep DVE
        2x mode.
  routing: per-partition free-dim ops only; tree reductions + multiplies
        on VectorE, exp/ln/square on ScalarE
        (squash scale = exp(0.5*ln(sq+eps) - ln(1+sq))).
"""

import math
import os
import sys
from concurrent.futures import ThreadPoolExecutor
from contextlib import ExitStack

import numpy as np

for _p in ("/opt/trn_rl_repo", "/opt/pypackages"):
    if _p not in sys.path and os.path.isdir(_p):
        sys.path.append(_p)

import concourse.bass as bass
import concourse.bacc as bacc
import concourse.tile as tile
from concourse import mybir

F32 = mybir.dt.float32
F16 = mybir.dt.float16
I8 = mybir.dt.int8
AF = mybir.ActivationFunctionType
OP = mybir.AluOpType

B, H, W_, IC, IA = 16, 64, 64, 8, 16
NCAP, NAT = 8, 16
KS, PAD = 5, 2
CORES = 8
HW = H * W_               # 4096
L = 512                   # conv chunk (one PSUM bank fp32)
NCK = HW // L             # 8 conv chunks
SEG = 32                  # capsule locations per channel row
TROW = H + 2 * PAD        # 68
TFREE = TROW * W_         # 4352
EPS = 1e-12
WBF = KS * 128 + 128      # packed weights+bias free dim = 768
OSCALE = 126.0            # int8 output quantization scale (margin below
                          # 127 so fp16 wobble can never wrap past +-127)

NSPLIT = 1                # calls per run (1: BPC=2; 2: BPC=1 pipelined)


def _build_program(bpc):
    ipc = bpc * IC            # conv images per core
    nc = bacc.Bacc(
        "TRN2",
        target_bir_lowering=False,
        debug=False,
        enable_asserts=False,
        num_devices=CORES,
    )
    xt = nc.dram_tensor("xt", [ipc, IA, H, W_], F16, kind="ExternalInput").ap()
    wb = nc.dram_tensor("wb", [128, WBF], F16, kind="ExternalInput").ap()
    out_d = nc.dram_tensor("out", [bpc, 128, HW], I8, kind="ExternalOutput").ap()

    with tile.TileContext(nc) as tc, ExitStack() as ctx:
        cpool = ctx.enter_context(tc.tile_pool(name="const", bufs=1))
        tpool = ctx.enter_context(tc.tile_pool(name="timg", bufs=2))
        big = ctx.enter_context(tc.tile_pool(name="big", bufs=2))
        one = ctx.enter_context(tc.tile_pool(name="one", bufs=1))
        ppool = ctx.enter_context(tc.tile_pool(name="ps", bufs=6, space="PSUM"))

        wb_sb = cpool.tile([128, WBF], F16, tag="wb")
        nc.sync.dma_start(wb_sb[:], wb)
        wl_sb = wb_sb[0:KS * IA, 0:KS * 128]          # [80, 640]
        biasr_sb = wb_sb[:, KS * 128:WBF]             # [128, 128]
        eps_sb = cpool.tile([128, 1], F32, tag="eps")
        nc.gpsimd.memset(eps_sb[:], EPS)
        one_sb = cpool.tile([128, 1], F32, tag="one")
        nc.gpsimd.memset(one_sb[:], 1.0)
        lsc_sb = cpool.tile([128, 1], F32, tag="lsc")
        nc.gpsimd.memset(lsc_sb[:], math.log(OSCALE))

        votes = cpool.tile([128, IC * HW], F16, tag="votes")
        out_sb = cpool.tile([128, HW], I8, tag="outsb")
        oq1 = cpool.tile([128, HW], F16, tag="oq1")
        a1 = cpool.tile([128, IC * SEG * NCAP], F16, tag="a1")   # [i, s, c]
        a2 = cpool.tile([128, IC * SEG * NCAP], F16, tag="a2")

        bias_bc = biasr_sb.unsqueeze(1).broadcast_to([128, SEG, 128])

        def vview(i):
            return votes[:, i * HW:(i + 1) * HW].rearrange(
                "p (s n c) -> p s n c", s=SEG, n=NAT)

        def snc(ap):
            return ap.rearrange("p (s n c) -> p s n c", s=SEG, n=NAT)

        def load_image(img):
            tb = tpool.tile([KS * IA, TFREE], F16, tag="tb")
            nc.gpsimd.memset(tb[:, 0:2 * W_].bitcast(F32), 0.0)
            nc.gpsimd.memset(tb[:, (TROW - 2) * W_:].bitcast(F32), 0.0)
            tv = tb[:].rearrange("p (r c) -> p r c", r=TROW)
            # zero edge columns on all partitions; valid DMAs overwrite
            nc.gpsimd.memset(tv[:, PAD:PAD + H, 0:PAD].bitcast(F32), 0.0)
            nc.gpsimd.memset(tv[:, PAD:PAD + H, W_ - PAD:W_].bitcast(F32), 0.0)
            for dx in range(KS):
                lo_dst = max(0, PAD - dx)
                lo_src = max(0, dx - PAD)
                cnt = W_ - abs(dx - PAD)
                nc.gpsimd.dma_start(
                    tv[dx * IA:(dx + 1) * IA, PAD:PAD + H, lo_dst:lo_dst + cnt],
                    xt[img, :, :, lo_src:lo_src + cnt],
                )
            return tb

        def conv_image(i, tb):
            for ck in range(NCK):
                ps = ppool.tile([128, L], F32, tag="conv")
                for dy in range(KS):
                    base = (8 * ck + dy) * W_
                    nc.tensor.matmul(
                        ps[:], wl_sb[:, dy * 128:(dy + 1) * 128],
                        tb[:, base:base + L],
                        start=(dy == 0), stop=(dy == KS - 1),
                        skip_group_check=True,
                    )
                dst = votes[:, i * HW + ck * L: i * HW + (ck + 1) * L]
                dv = dst.rearrange("p (s n c) -> p s n c", s=4, n=NAT)
                dperm = dv.transpose([0, 1, 3, 2])          # (s, c, n) order
                pv = ps[:].rearrange("p (s c n) -> p s c n", s=4, c=NCAP)
                nc.scalar.activation(dperm, pv, AF.Copy, scale=1.0)

        def tree_n(src4, dst_sc):
            """src4 [128, s, n, c] -> dst_sc [128, s*c] (sum over n)."""
            t1 = one.tile([128, SEG * 8 * NCAP], F16, tag="tn1")
            v1 = t1[:].rearrange("p (s n c) -> p s n c", s=SEG, n=8)
            nc.vector.tensor_add(v1, src4[:, :, 0:8, :], src4[:, :, 8:16, :])
            t2 = one.tile([128, SEG * 4 * NCAP], F16, tag="tn2")
            v2 = t2[:].rearrange("p (s n c) -> p s n c", s=SEG, n=4)
            nc.vector.tensor_add(v2, v1[:, :, 0:4, :], v1[:, :, 4:8, :])
            t3 = one.tile([128, SEG * 2 * NCAP], F16, tag="tn3")
            v3 = t3[:].rearrange("p (s n c) -> p s n c", s=SEG, n=2)
            nc.vector.tensor_add(v3, v2[:, :, 0:2, :], v2[:, :, 2:4, :])
            dv = dst_sc.rearrange("p (s c) -> p s c", s=SEG)
            nc.vector.tensor_add(dv, v3[:, :, 0, :], v3[:, :, 1, :])

        def squash(pcur, final=False):
            p2 = big.tile([128, HW], F16, tag="prod")
            nc.scalar.activation(p2[:], pcur[:], AF.Square)
            sq = one.tile([128, SEG * NCAP], F16, tag="sq")
            tree_n(snc(p2[:]), sq[:])
            la = one.tile([128, SEG * NCAP], F32, tag="la")
            nc.scalar.activation(la[:], sq[:], AF.Ln, bias=eps_sb[:])
            lb = one.tile([128, SEG * NCAP], F32, tag="lb")
            nc.scalar.activation(lb[:], sq[:], AF.Ln, bias=one_sb[:])
            st = one.tile([128, SEG * NCAP], F32, tag="st")
            nc.vector.scalar_tensor_tensor(
                out=st[:], in0=la[:], scalar=0.5, in1=lb[:],
                op0=OP.mult, op1=OP.subtract)
            sct = one.tile([128, SEG * NCAP], F16, tag="sct")
            if final:
                # fold the int8 quantization scale into the squash factor
                nc.scalar.activation(sct[:], st[:], AF.Exp, bias=lsc_sb[:])
                oq_t = oq1[:].rearrange(
                    "p (s c n) -> p s c n", s=SEG, c=NCAP).transpose([0, 1, 3, 2])
                scb = sct[:].rearrange("p (s c) -> p s c", s=SEG) \
                    .unsqueeze(2).broadcast_to([128, SEG, NAT, NCAP])
                nc.vector.tensor_mul(oq_t, snc(pcur[:]), scb)
                # exact round-to-nearest-integer in fp16: +1536 then -1536
                nc.scalar.activation(oq1[:], oq1[:], AF.Copy, bias=1536.0)
                nc.scalar.activation(out_sb[:], oq1[:], AF.Copy, bias=-1536.0)
                return None
            nc.scalar.activation(sct[:], st[:], AF.Exp)
            scb = sct[:].rearrange("p (s c) -> p s c", s=SEG) \
                .unsqueeze(2).broadcast_to([128, SEG, NAT, NCAP])
            act = one.tile([128, HW], F16, tag="act")
            nc.vector.tensor_mul(snc(act[:]), snc(pcur[:]), scb)
            return act

        def agreement(act, dst):
            """dst[:, i-block] = sum_n votes_i * act  (layout [i, s, c])."""
            ab = snc(act[:])
            for i in range(IC):
                prod = big.tile([128, HW], F16, tag="prod")
                eng = nc.gpsimd if i >= IC - 3 else nc.vector
                eng.tensor_mul(snc(prod[:]), vview(i), ab)
                tree_n(snc(prod[:]),
                       dst[:, i * SEG * NCAP:(i + 1) * SEG * NCAP])

        def softmax_preact(logits):
            """softmax over c of logits [128,(i,s,c)], route-weighted votes
            summed over i, + bias -> pcur tile."""
            lv = logits.rearrange("p (i s c) -> p i s c", i=IC, s=SEG)
            m1 = one.tile([128, IC * SEG * 4], F16, tag="m1")
            m1v = m1[:].rearrange("p (i s c) -> p i s c", i=IC, s=SEG)
            nc.vector.tensor_max(m1v, lv[:, :, :, 0:4], lv[:, :, :, 4:8])
            m2 = one.tile([128, IC * SEG * 2], F16, tag="m2")
            m2v = m2[:].rearrange("p (i s c) -> p i s c", i=IC, s=SEG)
            nc.vector.tensor_max(m2v, m1v[:, :, :, 0:2], m1v[:, :, :, 2:4])
            mm = one.tile([128, IC * SEG], F16, tag="mm")
            mmv = mm[:].rearrange("p (i s) -> p i s", i=IC)
            nc.vector.tensor_max(mmv, m2v[:, :, :, 0], m2v[:, :, :, 1])
            e = one.tile([128, IC * SEG * NCAP], F16, tag="e")
            ev = e[:].rearrange("p (i s c) -> p i s c", i=IC, s=SEG)
            mmb = mm[:].rearrange("p (i s) -> p i s", i=IC) \
                .unsqueeze(3).broadcast_to([128, IC, SEG, NCAP])
            nc.vector.tensor_sub(ev, lv, mmb)
            nc.scalar.activation(e[:], e[:], AF.Exp)
            c1 = one.tile([128, IC * SEG * 4], F16, tag="c1")
            c1v = c1[:].rearrange("p (i s c) -> p i s c", i=IC, s=SEG)
            nc.vector.tensor_add(c1v, ev[:, :, :, 0:4], ev[:, :, :, 4:8])
            c2 = one.tile([128, IC * SEG * 2], F16, tag="c2")
            c2v = c2[:].rearrange("p (i s c) -> p i s c", i=IC, s=SEG)
            nc.vector.tensor_add(c2v, c1v[:, :, :, 0:2], c1v[:, :, :, 2:4])
            se = one.tile([128, IC * SEG], F32, tag="se")
            sev = se[:].rearrange("p (i s) -> p i s", i=IC)
            nc.vector.tensor_add(sev, c2v[:, :, :, 0], c2v[:, :, :, 1])
            lr = one.tile([128, IC * SEG], F32, tag="lr")
            nc.scalar.activation(lr[:], se[:], AF.Ln)
            rr = one.tile([128, IC * SEG], F16, tag="rr")
            nc.scalar.activation(rr[:], lr[:], AF.Exp, scale=-1.0)
            rrb = rr[:].rearrange("p (i s) -> p i s", i=IC) \
                .unsqueeze(3).broadcast_to([128, IC, SEG, NCAP])
            nc.vector.tensor_mul(ev, ev, rrb)        # e becomes route
            pcur = one.tile([128, HW], F16, tag="pcur")
            rb0 = ev[:, 0].unsqueeze(2).broadcast_to([128, SEG, NAT, NCAP])
            nc.vector.tensor_mul(snc(pcur[:]), vview(0), rb0)
            for i in range(1, IC):
                wt = big.tile([128, HW], F16, tag="wb")
                rbi = ev[:, i].unsqueeze(2).broadcast_to([128, SEG, NAT, NCAP])
                eng = nc.gpsimd if i >= IC - 3 else nc.vector
                eng.tensor_mul(snc(wt[:]), vview(i), rbi)
                nc.vector.tensor_add(pcur[:], pcur[:], wt[:])
            pv = pcur[:].rearrange("p (s k) -> p s k", s=SEG)
            nc.vector.tensor_add(pv, pv, bias_bc)
            return pcur

        for bb in range(bpc):
            for i in range(IC):
                tb = load_image(bb * IC + i)
                conv_image(i, tb)
            # iter-1 preact: mean over input capsules of votes, + bias
            pc1 = one.tile([128, HW], F16, tag="pcur")
            nc.vector.tensor_add(pc1[:], votes[:, 0:HW], votes[:, HW:2 * HW])
            for i in range(2, IC):
                nc.vector.tensor_add(
                    pc1[:], pc1[:], votes[:, i * HW:(i + 1) * HW])
            p1v = pc1[:].rearrange("p (s k) -> p s k", s=SEG)
            nc.vector.scalar_tensor_tensor(
                out=p1v, in0=p1v, scalar=1.0 / IC, in1=bias_bc,
                op0=OP.mult, op1=OP.add)
            act = squash(pc1)
            agreement(act, a1[:])
            pc2 = softmax_preact(a1[:])
            act = squash(pc2)
            agreement(act, a2[:])
            nc.vector.tensor_add(a1[:], a1[:], a2[:])
            pc3 = softmax_preact(a1[:])
            squash(pc3, final=True)
            nc.sync.dma_start(out_d[bb], out_sb[:])

    nc.finalize()
    return nc


_CACHE = {}


class _Res:
    exec_time_ns = None
    results = None


def _make_compiled(bpc):
    import jax
    import jax.numpy as jnp
    from jax.sharding import Mesh, PartitionSpec, NamedSharding
    from jax.experimental.shard_map import shard_map
    from concourse.bass2jax import (
        _bass_exec_p, partition_id_tensor, install_neuronx_cc_hook)

    install_neuronx_cc_hook()
    nc = _build_program(bpc)

    partition_name = (
        nc.partition_id_tensor.name if nc.partition_id_tensor else None)
    in_names, out_names, out_avals = [], [], []
    for alloc in nc.m.functions[0].allocations:
        if not isinstance(alloc, mybir.MemoryLocationSet):
            continue
        name = alloc.memorylocations[0].name
        if alloc.kind == "ExternalInput":
            if name != partition_name:
                in_names.append(name)
        elif alloc.kind == "ExternalOutput":
            out_names.append(name)
            out_avals.append(jax.core.ShapedArray(
                tuple(alloc.tensor_shape), mybir.dt.np(alloc.dtype)))
    n_params = len(in_names)
    all_names = in_names + out_names + (
        [partition_name] if partition_name else [])

    def _body(*args):
        operands = list(args)
        if partition_name is not None:
            operands.append(partition_id_tensor())
        outs = _bass_exec_p.bind(
            *operands, out_avals=tuple(out_avals),
            in_names=tuple(all_names), out_names=tuple(out_names),
            lowering_input_output_aliases=(), sim_require_finite=True,
            sim_require_nnan=True, nc=nc)
        return tuple(outs)

    devices = jax.devices()[:CORES]
    _CACHE["devices"] = devices
    mesh = Mesh(np.asarray(devices), ("core",))
    n_all = n_params + len(out_names)
    sharded = jax.jit(
        shard_map(_body, mesh=mesh,
                  in_specs=(PartitionSpec("core"),) * n_all,
                  out_specs=(PartitionSpec("core"),) * len(out_names),
                  check_rep=False),
        keep_unused=True)

    arg_sds = [
        jax.ShapeDtypeStruct((CORES * bpc * IC, IA, H, W_), np.float16),
        jax.ShapeDtypeStruct((CORES * 128, WBF), np.float16),
        jax.ShapeDtypeStruct((CORES * bpc, 128, HW), np.int8),
    ]
    try:
        compiled = sharded.lower(*arg_sds).compile()
    except Exception:
        zero_args = [np.zeros(s.shape, s.dtype) for s in arg_sds]
        compiled = sharded.lower(*zero_args).compile()

    sh = NamedSharding(mesh, PartitionSpec("core"))
    _CACHE["sh"] = sh
    # persistent output-init operand: the kernel writes every output
    # element, so the content never matters and it is never donated.
    oinit = jax.jit(
        lambda: jnp.zeros((CORES * bpc, 128, HW), jnp.int8),
        out_shardings=sh)()
    jax.block_until_ready(oinit)
    return compiled, oinit


def _ensure_compiled():
    if "compiled" not in _CACHE:
        _CACHE["compiled"], _CACHE["oinit"] = _make_compiled(B // CORES // NSPLIT)
        _CACHE["pool"] = ThreadPoolExecutor(CORES)
    return _CACHE


def _fetch_deq(out, buf):
    """Per-shard D2H in parallel threads: each shard fetch costs a ~90 ms
    round trip on the axon tunnel, so 8 serial fetches (or one aggregate
    np.asarray) are latency-bound; threads overlap them and also hide the
    device exec still in flight when the requests are issued. Each thread
    also runs the flake guard and int8->f32 dequantization on its shard,
    so that work hides inside the other shards' fetch latency.

    Returns True if any shard looks like the rare first-touch flake:
    -128 can only appear if the device produced NaN/garbage (|act|*126 <
    127), and a legitimately all-zero group row is impossible."""
    def grab(s):
        d = np.asarray(s.data)
        bad = bool((d == -128).any()) or not d.any(axis=(1, 2)).all()
        buf[s.index] = d.astype(np.float32)
        buf[s.index] *= 1.0 / OSCALE
        return bad
    return any(_CACHE["pool"].map(grab, out.addressable_shards))


def _host_wb(W, b):
    W = np.asarray(W, np.float32)
    b = np.asarray(b, np.float32)
    wb = np.zeros((128, WBF), np.float16)
    wb[0:KS * IA, 0:KS * 128] = np.ascontiguousarray(
        W.transpose(3, 1, 2, 0)).reshape(KS * IA, KS * 128)
    bp = b.reshape(NCAP, NAT).T.reshape(128)       # (atom, cap) order
    wb[:, KS * 128:] = bp[None, :].astype(np.float16)
    return np.tile(wb, (CORES, 1))


def _host_xt(x, s):
    # torch-style reinterpret: (ic,Bs,H,W,ia) row-major -> (ic*Bs, ia, H, W)
    # part s of NSPLIT covers x batches [Bs*s, Bs*(s+1)); its row r maps to
    # conv image 16k + (B//CORES//NSPLIT)*IC ... laid out core-major.
    bs = B // NSPLIT
    return np.asarray(x, np.float32)[bs * s:bs * (s + 1)] \
        .transpose(3, 0, 1, 2, 4).astype(np.float16).reshape(
            IC * bs, IA, H, W_)


def _push_xt(x, st):
    """Per-core shard prep + H2D push in parallel threads. Core k's 16 conv
    images are exactly x[:, :, :, k, :] under the torch-style reinterpret,
    so each thread does a ~4 ms slice-cast and immediately starts its wire
    transfer instead of waiting for the full 35 ms host transpose."""
    import jax
    x = np.asarray(x, np.float32)

    def prep_push(k):
        xs = x[:, :, :, k, :].astype(np.float16).reshape(B, IA, H, W_)
        return jax.device_put(xs, st["devices"][k])

    bufs = list(_CACHE["pool"].map(prep_push, range(CORES)))
    return jax.make_array_from_single_device_arrays(
        (CORES * B, IA, H, W_), st["sh"], bufs)


def _wb_dev(W, b):
    """Device-resident packed weights+bias, content-keyed (weights are
    stable across calls, so the ~1.5 MB push amortizes away)."""
    import jax
    key = (np.asarray(W, np.float32).tobytes(),
           np.asarray(b, np.float32).tobytes())
    if _CACHE.get("wb_key") != key:
        dev = jax.device_put(_host_wb(W, b), _CACHE["sh"])
        jax.block_until_ready(dev)
        _CACHE["wb_dev"] = dev
        _CACHE["wb_key"] = key
    return _CACHE["wb_dev"]


def run(x, W, b, trace=False, **kw):
    st = _ensure_compiled()
    wbg = _wb_dev(W, b)
    buf = np.empty((B, 128, HW), np.float32)
    for _attempt in range(2):
        if NSPLIT == 1:
            out = st["compiled"](_push_xt(x, st), wbg, st["oinit"])[0]
            if not _fetch_deq(out, buf):
                break
        else:
            parts = [st["compiled"](_host_xt(x, s), wbg, st["oinit"])[0]
                     for s in range(NSPLIT)]
            if not any(_fetch_deq(p, buf[s::NSPLIT])
                       for s, p in enumerate(parts)):
                break
    full = buf.reshape(B, 128, SEG, NCAP, NAT)
    full = full.reshape(B, HW, NCAP, NAT).reshape(B, H, W_, NCAP, NAT)
    return full, _Res()


def kernel(x, W, b):
    out, _ = run(x, W, b, trace=False)
    return out
